# revision 15
# baseline (speedup 1.0000x reference)
"""Trainium2 Bass kernel for nn_Attention_27977416966318 (sparse_attention).

score[b,s] = v . tanh(W @ concat(static[b,s], dynamic[b,s], dec[b]))
out = softmax(score, axis=1)

Shapes: static/dynamic [64, 2048, 256] f32, decoder_hidden [64, 256],
v [1, 768], W [768, 768].  Output [64, 2048] f32.

Strategy: data-parallel over batch B=64 across 8 NeuronCores (8 batches
per core).  The dominant matmul W12 @ [static; dynamic] runs in fp8
(e4m3) with DoubleRow perf mode.  W12 is pre-scaled by 32 before fp8
quantization; the tanh activation applies scale=1/32 to undo it and
adds the per-batch decoder bias (host-computed W3 @ dec).

The kernel is dual-bottlenecked on PE (fp8-DR matmul stream) and ACT
(tanh on 768*2048 elems/batch).  vs the previous version:
  - tanh tiles are [128, 2048] (PSUM pair of h buffers = all 8 banks),
    halving the ~540-cycle per-ACTIVATE overhead contribution.
  - exp moved to the host (device ships raw score strips; softmax is
    computed host-side in f64) - saves ACT time and the tail.
  - the v-dot uses 2-row x 4-col array tiling (8 concurrent K=64
    tiles); the two row-half partial scores are summed on the host.
  - the score accumulator rides the h PSUM pool ring (same tag), so
    no PSUM bank is reserved for it.

fp8 quantization error is tamed by a rank-1 linearization correction
computed on the host:

    corr = ALPHA * ((v^T W12) @ x  -  (v^T W8/32) @ x8)

where x8/W8 are the exact fp8 operand values the PE sees.  Writing
score = v.tanh(h~) + ALPHA*v.(h - h~), the residual sensitivity is
(tanh'(h) - ALPHA); ALPHA=0.55 ~ E[tanh'] minimizes it (rel err
~1.25e-2, gate 2e-2).  The host adds corr to the raw scores before
its softmax.
"""

import os

import numpy as np
import ml_dtypes

import concourse.bass as bass
from concourse import bacc
import concourse.mybir as mybir
import concourse.tile as tile
from concourse.bass_utils import run_bass_kernel_spmd

B, S, H = 64, 2048, 256
H3 = 3 * H          # 768
NCORES = 8
BL = B // NCORES    # 8 batches per core
T = BL * S          # 16384 tokens per core
KT = 4              # contraction k-tiles of 128 (2 static + 2 dynamic)
MT = H3 // 128      # 6 output o-tiles
WSCALE = 32.0       # fp8 pre-scale on W12
ALPHA = 0.55        # linearization coefficient
F32 = mybir.dt.float32
BF16 = mybir.dt.bfloat16
FP8 = mybir.dt.float8e4
TANH = mybir.ActivationFunctionType.Tanh
DR = mybir.MatmulPerfMode.DoubleRow

_CACHED = {}


def build_bass():
    nc = bacc.Bacc(None, target_bir_lowering=False, debug=False)
    x = nc.dram_tensor("x8_t", [2 * H, T], FP8, kind="ExternalInput")
    xr = x.rearrange("(t p) n -> p t n", p=128)  # [128, 4, T]
    wt8 = nc.dram_tensor("wt8", [2 * H, H3], FP8, kind="ExternalInput")
    # host pre-transposes bias/v into per-partition-contiguous layouts so
    # their DMAs are a few large packets, not 768 tiny strided ones
    biasx = nc.dram_tensor("biasx", [128, MT * BL], F32, kind="ExternalInput")
    vv = nc.dram_tensor("v", [128, MT], BF16, kind="ExternalInput")
    # per b: 4 col strips x (two row-half partial scores, 512 tokens each);
    # host sums the halves, adds corr, softmaxes.
    out = nc.dram_tensor("out", [BL, 4, 1024], F32, kind="ExternalOutput")

    with tile.TileContext(nc) as tc:
        with (
            tc.tile_pool(name="const", bufs=1) as constp,
            tc.tile_pool(name="xp", bufs=2) as xp,
            tc.tile_pool(name="thp", bufs=15) as thp,
            tc.tile_pool(name="hps", bufs=4, space="PSUM") as hps,
        ):
            # ---- PE warmup: a few zero matmuls bridge the gap until the
            # first x/wt8 DMAs land (and start the HAM busy window).  The
            # warm tile is the first allocation of the h ring. ----
            warm = constp.tile([1, 512], BF16)
            nc.gpsimd.memset(warm, 0.0)
            warm_ps = hps.tile([128, 1024], F32, tag="h", name="warm_ps")
            for i in range(7):
                nc.tensor.matmul(
                    out=warm_ps[:, 0:512], lhsT=warm[:, 0:128], rhs=warm,
                    start=True, stop=True,
                )

            # ---- startup DMAs, balanced across both rings.  b0's x is
            # fetched as 4 whole k-tile stripes ([128, 1, 2048] each ->
            # 2048B contiguous lines, twice the packet efficiency of
            # token-chunk splits).  bias/v first (tiny, gate the first
            # tanh); wt8 k-tiles split 2+2.
            # wt8 is 32*W12.T quantized e4m3: [k 0..511, o]; k-tiles 0-1
            # static, 2-3 dynamic.
            wt8r = wt8.rearrange("(t p) o -> p t o", p=128)
            wt8_sb = constp.tile([128, KT, H3], FP8)
            first_xt = xp.tile([128, KT, S], FP8, tag="x", name="x_0")
            v_sb = constp.tile([128, MT], BF16)
            bias_sb = constp.tile([128, MT * BL], F32)

            nc.scalar.dma_start(out=bias_sb, in_=biasx[:, :])
            nc.scalar.dma_start(out=v_sb, in_=vv[:, :])
            nc.sync.dma_start(out=wt8_sb[:, 2:3, :], in_=wt8r[:, 2:3, :])
            nc.scalar.dma_start(out=wt8_sb[:, 0:1, :], in_=wt8r[:, 0:1, :])
            nc.sync.dma_start(out=wt8_sb[:, 3:4, :], in_=wt8r[:, 3:4, :])
            nc.scalar.dma_start(out=wt8_sb[:, 1:2, :], in_=wt8r[:, 1:2, :])
            nc.sync.dma_start(out=first_xt[:, 0:1, :], in_=xr[:, 0:1, 0:S])
            nc.scalar.dma_start(out=first_xt[:, 2:3, :], in_=xr[:, 2:3, 0:S])
            nc.sync.dma_start(out=first_xt[:, 1:2, :], in_=xr[:, 1:2, 0:S])
            nc.scalar.dma_start(out=first_xt[:, 3:4, :], in_=xr[:, 3:4, 0:S])

            # ---- main loop ----
            # v-dot for batch b is emitted after batch b+1's first matmul
            # block so the PE never idles waiting on b's last tanh
            pending = []

            def flush_pending():
                for emit in pending:
                    emit()
                pending.clear()

            for b in range(BL):
                ths = {}
                if b == 0:
                    xt = first_xt
                else:
                    xt = xp.tile([128, KT, S], FP8, tag="x", name=f"x_{b}")
                    nc.sync.dma_start(out=xt, in_=xr[:, :, b * S : (b + 1) * S])
                for m in range(MT):
                    for hf in range(2):
                        if b > 0 and m == 1 and hf == 0:
                            flush_pending()
                        h_ps = hps.tile(
                            [128, 1024], F32, tag="h", name=f"h_{b}_{m}_{hf}"
                        )
                        for c2 in range(2):
                            lo = (2 * hf + c2) * 512
                            for t_i in range(2):
                                nc.tensor.matmul(
                                    out=h_ps[:, c2 * 512 : (c2 + 1) * 512],
                                    lhsT=wt8_sb[:, 2 * t_i : 2 * t_i + 2,
                                                m * 128 : (m + 1) * 128],
                                    rhs=xt[:, 2 * t_i : 2 * t_i + 2, lo : lo + 512],
                                    start=(t_i == 0),
                                    stop=(t_i == 1),
                                    perf_mode=DR,
                                )
                        th = thp.tile(
                            [128, 1024], BF16, tag="tanh", name=f"th_{b}_{m}_{hf}"
                        )
                        nc.scalar.activation(
                            out=th, in_=h_ps, func=TANH,
                            scale=1.0 / WSCALE,
                            bias=bias_sb[:, m * BL + b : m * BL + b + 1],
                        )
                        ths[(m, hf)] = th

                def emit_scores(b=b, ths=ths):
                    # 2-row x 4-col array tiling: 8 concurrent K=64 tiles.
                    # Strip (a, c): contraction rows 64a..64a+63, output at
                    # partition 32c, free range 512a..512a+512.
                    score = hps.tile([128, 1024], F32, tag="h", name=f"sc_{b}")
                    for m in range(MT):
                        for a in range(2):
                            for c in range(4):
                                nc.tensor.matmul(
                                    out=score[32 * c : 32 * c + 1,
                                              512 * a : 512 * a + 512],
                                    lhsT=v_sb[64 * a : 64 * a + 64, m : m + 1],
                                    rhs=ths[(m, c // 2)][
                                        64 * a : 64 * a + 64,
                                        (c % 2) * 512 : (c % 2 + 1) * 512],
                                    start=(m == 0),
                                    stop=(m == MT - 1),
                                    tile_position=(64 * a, 32 * c),
                                )
                    # raw score strips PSUM -> SBUF (idle DVE, full-width --
                    # engines can't stride partitions) -> HBM (DMA can);
                    # host does half-sum + corr + softmax
                    stage = thp.tile(
                        [128, 1024], F32, tag="stage", bufs=2, name=f"stage_{b}"
                    )
                    if b == BL - 1:
                        # tail: split across DVE + ACT (ACT is idle after the
                        # last tanh) to shorten the drain chain
                        nc.vector.tensor_copy(stage[:, 0:512], score[:, 0:512])
                        nc.scalar.copy(stage[:, 512:1024], score[:, 512:1024])
                    else:
                        nc.vector.tensor_copy(stage, score)
                    sview = stage.rearrange("(c r) f -> c r f", c=4)[:, 0, :]
                    nc.scalar.dma_start(out=out[b], in_=sview)

                pending.append(emit_scores)
            flush_pending()

    nc.compile()
    return nc


def kernel(static, dynamic, decoder_hidden, v, W):
    static = np.ascontiguousarray(np.asarray(static, dtype=np.float32))
    dynamic = np.ascontiguousarray(np.asarray(dynamic, dtype=np.float32))
    decoder_hidden = np.ascontiguousarray(np.asarray(decoder_hidden, dtype=np.float32))
    v = np.ascontiguousarray(np.asarray(v, dtype=np.float32))
    W = np.ascontiguousarray(np.asarray(W, dtype=np.float32))

    bf16 = ml_dtypes.bfloat16
    e4m3 = ml_dtypes.float8_e4m3

    W12 = W[:, : 2 * H]                       # [768, 512]
    wt8 = np.ascontiguousarray(W12.T * WSCALE).astype(e4m3)   # [512, 768]
    # per-batch decoder bias (bf16 operands, f32 accumulate -- matches the
    # PE bias path of the bf16 baseline)
    W3f = W[:, 2 * H :].astype(bf16).astype(np.float32)        # [768, 256]
    decf = decoder_hidden.astype(bf16).astype(np.float32)      # [B, 256]
    bias_all = W3f @ decf.T                                    # [768, B]

    # linearization correction (host): ALPHA * (x @ u_lin - x8 @ u8)
    u_lin = W12.astype(np.float64).T @ v[0].astype(np.float64)          # [512]
    u8 = (wt8.astype(np.float64) / WSCALE) @ v[0].astype(np.float64)    # [512]

    in_maps = []
    corrs = []
    for c in range(NCORES):
        sl = slice(c * BL, (c + 1) * BL)
        xc = np.empty((T, 2 * H), dtype=np.float32)
        xc[:, :H] = static[sl].reshape(T, H)
        xc[:, H:] = dynamic[sl].reshape(T, H)
        x8 = xc.astype(e4m3)                         # [T, 512]
        x8f = x8.astype(np.float32)
        corr = ALPHA * (
            xc @ u_lin.astype(np.float32) - x8f @ u8.astype(np.float32)
        )                                            # [T]
        x8_t = np.ascontiguousarray(x8.T)            # [512, T]
        corrs.append(corr)
        # bias/v pre-transposed to per-partition-contiguous [128, m*BL+b]
        # and [128, m] layouts (o = m*128 + p)
        bias_pt = np.ascontiguousarray(
            bias_all[:, sl].reshape(MT, 128, BL).transpose(1, 0, 2).reshape(128, -1)
        )
        v_pt = np.ascontiguousarray(v[0].reshape(MT, 128).T.astype(bf16))
        in_maps.append({
            "x8_t": x8_t, "wt8": wt8,
            "biasx": bias_pt,
            "v": v_pt,
        })

    if "nc" not in _CACHED:
        _CACHED["nc"] = build_bass()
    nc = _CACHED["nc"]

    trace = bool(int(os.environ.get("KERNEL_TRACE", "0")))
    res = run_bass_kernel_spmd(
        nc, in_maps, core_ids=list(range(NCORES)), trace=trace,
        trace_cores=[0] if trace else None,
    )
    _CACHED["last_result"] = res

    raw = np.concatenate([r["out"] for r in res.results], axis=0)  # [B, 4, 1024]
    s = raw[:, :, :512].astype(np.float64) + raw[:, :, 512:].astype(np.float64)
    z = s.reshape(B, S) + np.concatenate(corrs).reshape(B, S).astype(np.float64)
    z -= z.max(axis=1, keepdims=True)
    ez = np.exp(z)
    return (ez / ez.sum(axis=1, keepdims=True)).astype(np.float32)


# revision 17
# speedup vs baseline: 1.0118x; 1.0118x over previous
"""Trainium2 Bass kernel for nn_Attention_27977416966318 (sparse_attention).

score[b,s] = v . tanh(W @ concat(static[b,s], dynamic[b,s], dec[b]))
out = softmax(score, axis=1)

Shapes: static/dynamic [64, 2048, 256] f32, decoder_hidden [64, 256],
v [1, 768], W [768, 768].  Output [64, 2048] f32.

Strategy: data-parallel over batch B=64 across 8 NeuronCores (8 batches
per core).  The dominant matmul W12 @ [static; dynamic] runs in fp8
(e4m3) with DoubleRow perf mode.  W12 is pre-scaled by 32 before fp8
quantization; the tanh activation applies scale=1/32 to undo it and
adds the per-batch decoder bias (host-computed W3 @ dec).

The kernel is dual-bottlenecked on PE (fp8-DR matmul stream) and ACT
(tanh on 768*2048 elems/batch).  vs the previous version:
  - tanh tiles are [128, 2048] (PSUM pair of h buffers = all 8 banks),
    halving the ~540-cycle per-ACTIVATE overhead contribution.
  - exp moved to the host (device ships raw score strips; softmax is
    computed host-side in f64) - saves ACT time and the tail.
  - the v-dot uses 2-row x 4-col array tiling (8 concurrent K=64
    tiles); the two row-half partial scores are summed on the host.
  - the score accumulator rides the h PSUM pool ring (same tag), so
    no PSUM bank is reserved for it.

fp8 quantization error is tamed by a rank-1 linearization correction
computed on the host:

    corr = ALPHA * ((v^T W12) @ x  -  (v^T W8/32) @ x8)

where x8/W8 are the exact fp8 operand values the PE sees.  Writing
score = v.tanh(h~) + ALPHA*v.(h - h~), the residual sensitivity is
(tanh'(h) - ALPHA); ALPHA=0.55 ~ E[tanh'] minimizes it (rel err
~1.25e-2, gate 2e-2).  The host adds corr to the raw scores before
its softmax.
"""

import os

import numpy as np
import ml_dtypes

import concourse.bass as bass
from concourse import bacc
import concourse.mybir as mybir
import concourse.tile as tile
from concourse.bass_utils import run_bass_kernel_spmd

B, S, H = 64, 2048, 256
H3 = 3 * H          # 768
NCORES = 8
BL = B // NCORES    # 8 batches per core
T = BL * S          # 16384 tokens per core
KT = 4              # contraction k-tiles of 128 (2 static + 2 dynamic)
MT = H3 // 128      # 6 output o-tiles
WSCALE = 32.0       # fp8 pre-scale on W12
ALPHA = 0.55        # linearization coefficient
F32 = mybir.dt.float32
BF16 = mybir.dt.bfloat16
FP8 = mybir.dt.float8e4
TANH = mybir.ActivationFunctionType.Tanh
DR = mybir.MatmulPerfMode.DoubleRow

_CACHED = {}


def build_bass():
    nc = bacc.Bacc(None, target_bir_lowering=False, debug=False)
    x = nc.dram_tensor("x8_t", [2 * H, T], FP8, kind="ExternalInput")
    xr = x.rearrange("(t p) n -> p t n", p=128)  # [128, 4, T]
    wt8 = nc.dram_tensor("wt8", [2 * H, H3], FP8, kind="ExternalInput")
    # host pre-transposes bias/v into per-partition-contiguous layouts so
    # their DMAs are a few large packets, not 768 tiny strided ones
    biasx = nc.dram_tensor("biasx", [128, MT * BL], F32, kind="ExternalInput")
    vv = nc.dram_tensor("v", [128, MT], BF16, kind="ExternalInput")
    # per b: 4 col strips x (two row-half partial scores, 512 tokens each);
    # host sums the halves, adds corr, softmaxes.
    out = nc.dram_tensor("out", [BL, 4, 1024], F32, kind="ExternalOutput")

    with tile.TileContext(nc) as tc:
        with (
            tc.tile_pool(name="const", bufs=1) as constp,
            tc.tile_pool(name="xp", bufs=2) as xp,
            tc.tile_pool(name="thp", bufs=15) as thp,
            tc.tile_pool(name="hps", bufs=4, space="PSUM") as hps,
        ):
            # ---- PE warmup: a few zero matmuls bridge the gap until the
            # first x/wt8 DMAs land (and start the HAM busy window).  The
            # warm tile is the first allocation of the h ring. ----
            warm = constp.tile([1, 512], BF16)
            nc.gpsimd.memset(warm, 0.0)
            warm_ps = hps.tile([128, 1024], F32, tag="h", name="warm_ps")
            for i in range(7):
                nc.tensor.matmul(
                    out=warm_ps[:, 0:512], lhsT=warm[:, 0:128], rhs=warm,
                    start=True, stop=True,
                )

            # ---- startup DMAs, balanced across both rings.  b0's x is
            # fetched as 4 whole k-tile stripes ([128, 1, 2048] each ->
            # 2048B contiguous lines, twice the packet efficiency of
            # token-chunk splits).  bias/v first (tiny, gate the first
            # tanh); wt8 k-tiles split 2+2.
            # wt8 is 32*W12.T quantized e4m3: [k 0..511, o]; k-tiles 0-1
            # static, 2-3 dynamic.
            wt8r = wt8.rearrange("(t p) o -> p t o", p=128)
            wt8_sb = constp.tile([128, KT, H3], FP8)
            first_xt = xp.tile([128, KT, S], FP8, tag="x", name="x_0")
            v_sb = constp.tile([128, MT], BF16)
            bias_sb = constp.tile([128, MT * BL], F32)

            nc.scalar.dma_start(out=bias_sb, in_=biasx[:, :])
            nc.scalar.dma_start(out=v_sb, in_=vv[:, :])
            nc.sync.dma_start(out=wt8_sb[:, 2:3, :], in_=wt8r[:, 2:3, :])
            nc.scalar.dma_start(out=wt8_sb[:, 0:1, :], in_=wt8r[:, 0:1, :])
            nc.sync.dma_start(out=wt8_sb[:, 3:4, :], in_=wt8r[:, 3:4, :])
            nc.scalar.dma_start(out=wt8_sb[:, 1:2, :], in_=wt8r[:, 1:2, :])
            # b0 x in [kt-pair x token-half] quarters (1024B lines); each
            # ring carries one k-half of the first tanh's tokens
            nc.sync.dma_start(out=first_xt[:, 0:2, 0:1024], in_=xr[:, 0:2, 0:1024])
            nc.scalar.dma_start(out=first_xt[:, 2:4, 0:1024], in_=xr[:, 2:4, 0:1024])
            nc.sync.dma_start(
                out=first_xt[:, 0:2, 1024:2048], in_=xr[:, 0:2, 1024:2048]
            )
            nc.scalar.dma_start(
                out=first_xt[:, 2:4, 1024:2048], in_=xr[:, 2:4, 1024:2048]
            )

            # ---- main loop ----
            # v-dot for batch b is emitted after batch b+1's first matmul
            # block so the PE never idles waiting on b's last tanh
            pending = []

            def flush_pending():
                for emit in pending:
                    emit()
                pending.clear()

            next_xt = {}
            for b in range(BL):
                ths = {}
                if b == 0:
                    xt = first_xt
                else:
                    xt = next_xt.pop(b)
                for m in range(MT):
                    for hf in range(2):
                        if b > 0 and m == 1 and hf == 0:
                            flush_pending()
                        if m == 3 and hf == 0 and b + 1 < BL:
                            # prefetch next batch's x mid-batch: late enough
                            # not to starve startup/steady DMAs, early enough
                            # to land before b+1 needs it
                            nxt = xp.tile(
                                [128, KT, S], FP8, tag="x", name=f"x_{b + 1}"
                            )
                            nc.sync.dma_start(
                                out=nxt, in_=xr[:, :, (b + 1) * S : (b + 2) * S]
                            )
                            next_xt[b + 1] = nxt
                        h_ps = hps.tile(
                            [128, 1024], F32, tag="h", name=f"h_{b}_{m}_{hf}"
                        )
                        for c2 in range(2):
                            lo = (2 * hf + c2) * 512
                            for t_i in range(2):
                                nc.tensor.matmul(
                                    out=h_ps[:, c2 * 512 : (c2 + 1) * 512],
                                    lhsT=wt8_sb[:, 2 * t_i : 2 * t_i + 2,
                                                m * 128 : (m + 1) * 128],
                                    rhs=xt[:, 2 * t_i : 2 * t_i + 2, lo : lo + 512],
                                    start=(t_i == 0),
                                    stop=(t_i == 1),
                                    perf_mode=DR,
                                )
                        th = thp.tile(
                            [128, 1024], BF16, tag="tanh", name=f"th_{b}_{m}_{hf}"
                        )
                        nc.scalar.activation(
                            out=th, in_=h_ps, func=TANH,
                            scale=1.0 / WSCALE,
                            bias=bias_sb[:, m * BL + b : m * BL + b + 1],
                        )
                        ths[(m, hf)] = th

                def emit_scores(b=b, ths=ths):
                    # 2-row x 4-col array tiling: 8 concurrent K=64 tiles.
                    # Strip (a, c): contraction rows 64a..64a+63, output at
                    # partition 32c, free range 512a..512a+512.
                    score = hps.tile([128, 1024], F32, tag="h", name=f"sc_{b}")
                    for m in range(MT):
                        for a in range(2):
                            for c in range(4):
                                nc.tensor.matmul(
                                    out=score[32 * c : 32 * c + 1,
                                              512 * a : 512 * a + 512],
                                    lhsT=v_sb[64 * a : 64 * a + 64, m : m + 1],
                                    rhs=ths[(m, c // 2)][
                                        64 * a : 64 * a + 64,
                                        (c % 2) * 512 : (c % 2 + 1) * 512],
                                    start=(m == 0),
                                    stop=(m == MT - 1),
                                    tile_position=(64 * a, 32 * c),
                                )
                    # raw score strips PSUM -> SBUF (idle DVE, full-width --
                    # engines can't stride partitions) -> HBM (DMA can);
                    # host does half-sum + corr + softmax
                    stage = thp.tile(
                        [128, 1024], F32, tag="stage", bufs=2, name=f"stage_{b}"
                    )
                    if b == BL - 1:
                        # tail: split across DVE + ACT (ACT is idle after the
                        # last tanh) to shorten the drain chain
                        nc.vector.tensor_copy(stage[:, 0:512], score[:, 0:512])
                        nc.scalar.copy(stage[:, 512:1024], score[:, 512:1024])
                    else:
                        nc.vector.tensor_copy(stage, score)
                    sview = stage.rearrange("(c r) f -> c r f", c=4)[:, 0, :]
                    nc.scalar.dma_start(out=out[b], in_=sview)

                pending.append(emit_scores)
            flush_pending()

    nc.compile()
    return nc


def kernel(static, dynamic, decoder_hidden, v, W):
    static = np.ascontiguousarray(np.asarray(static, dtype=np.float32))
    dynamic = np.ascontiguousarray(np.asarray(dynamic, dtype=np.float32))
    decoder_hidden = np.ascontiguousarray(np.asarray(decoder_hidden, dtype=np.float32))
    v = np.ascontiguousarray(np.asarray(v, dtype=np.float32))
    W = np.ascontiguousarray(np.asarray(W, dtype=np.float32))

    bf16 = ml_dtypes.bfloat16
    e4m3 = ml_dtypes.float8_e4m3

    W12 = W[:, : 2 * H]                       # [768, 512]
    wt8 = np.ascontiguousarray(W12.T * WSCALE).astype(e4m3)   # [512, 768]
    # per-batch decoder bias (bf16 operands, f32 accumulate -- matches the
    # PE bias path of the bf16 baseline)
    W3f = W[:, 2 * H :].astype(bf16).astype(np.float32)        # [768, 256]
    decf = decoder_hidden.astype(bf16).astype(np.float32)      # [B, 256]
    bias_all = W3f @ decf.T                                    # [768, B]

    # linearization correction (host): ALPHA * (x @ u_lin - x8 @ u8)
    u_lin = W12.astype(np.float64).T @ v[0].astype(np.float64)          # [512]
    u8 = (wt8.astype(np.float64) / WSCALE) @ v[0].astype(np.float64)    # [512]

    in_maps = []
    corrs = []
    for c in range(NCORES):
        sl = slice(c * BL, (c + 1) * BL)
        xc = np.empty((T, 2 * H), dtype=np.float32)
        xc[:, :H] = static[sl].reshape(T, H)
        xc[:, H:] = dynamic[sl].reshape(T, H)
        x8 = xc.astype(e4m3)                         # [T, 512]
        x8f = x8.astype(np.float32)
        corr = ALPHA * (
            xc @ u_lin.astype(np.float32) - x8f @ u8.astype(np.float32)
        )                                            # [T]
        x8_t = np.ascontiguousarray(x8.T)            # [512, T]
        corrs.append(corr)
        # bias/v pre-transposed to per-partition-contiguous [128, m*BL+b]
        # and [128, m] layouts (o = m*128 + p)
        bias_pt = np.ascontiguousarray(
            bias_all[:, sl].reshape(MT, 128, BL).transpose(1, 0, 2).reshape(128, -1)
        )
        v_pt = np.ascontiguousarray(v[0].reshape(MT, 128).T.astype(bf16))
        in_maps.append({
            "x8_t": x8_t, "wt8": wt8,
            "biasx": bias_pt,
            "v": v_pt,
        })

    if "nc" not in _CACHED:
        _CACHED["nc"] = build_bass()
    nc = _CACHED["nc"]

    trace = bool(int(os.environ.get("KERNEL_TRACE", "0")))
    res = run_bass_kernel_spmd(
        nc, in_maps, core_ids=list(range(NCORES)), trace=trace,
        trace_cores=[0] if trace else None,
    )
    _CACHED["last_result"] = res

    raw = np.concatenate([r["out"] for r in res.results], axis=0)  # [B, 4, 1024]
    s = raw[:, :, :512].astype(np.float64) + raw[:, :, 512:].astype(np.float64)
    z = s.reshape(B, S) + np.concatenate(corrs).reshape(B, S).astype(np.float64)
    z -= z.max(axis=1, keepdims=True)
    ez = np.exp(z)
    return (ez / ez.sum(axis=1, keepdims=True)).astype(np.float32)


# revision 18
# speedup vs baseline: 1.0277x; 1.0157x over previous
"""Trainium2 Bass kernel for nn_Attention_27977416966318 (sparse_attention).

score[b,s] = v . tanh(W @ concat(static[b,s], dynamic[b,s], dec[b]))
out = softmax(score, axis=1)

Shapes: static/dynamic [64, 2048, 256] f32, decoder_hidden [64, 256],
v [1, 768], W [768, 768].  Output [64, 2048] f32.

Strategy: data-parallel over batch B=64 across 8 NeuronCores (8 batches
per core).  The dominant matmul W12 @ [static; dynamic] runs in fp8
(e4m3) with DoubleRow perf mode.  W12 is pre-scaled by 32 before fp8
quantization; the tanh activation applies scale=1/32 to undo it and
adds the per-batch decoder bias (host-computed W3 @ dec).

The kernel is dual-bottlenecked on PE (fp8-DR matmul stream) and ACT
(tanh on 768*2048 elems/batch).  vs the previous version:
  - tanh tiles are [128, 2048] (PSUM pair of h buffers = all 8 banks),
    halving the ~540-cycle per-ACTIVATE overhead contribution.
  - exp moved to the host (device ships raw score strips; softmax is
    computed host-side in f64) - saves ACT time and the tail.
  - the v-dot uses 2-row x 4-col array tiling (8 concurrent K=64
    tiles); the two row-half partial scores are summed on the host.
  - the score accumulator rides the h PSUM pool ring (same tag), so
    no PSUM bank is reserved for it.

fp8 quantization error is tamed by a rank-1 linearization correction
computed on the host:

    corr = ALPHA * ((v^T W12) @ x  -  (v^T W8/32) @ x8)

where x8/W8 are the exact fp8 operand values the PE sees.  Writing
score = v.tanh(h~) + ALPHA*v.(h - h~), the residual sensitivity is
(tanh'(h) - ALPHA); ALPHA=0.55 ~ E[tanh'] minimizes it (rel err
~1.25e-2, gate 2e-2).  The host adds corr to the raw scores before
its softmax.
"""

import os

import numpy as np
import ml_dtypes

import concourse.bass as bass
from concourse import bacc
import concourse.mybir as mybir
import concourse.tile as tile
from concourse.bass_utils import run_bass_kernel_spmd

B, S, H = 64, 2048, 256
H3 = 3 * H          # 768
NCORES = 8
BL = B // NCORES    # 8 batches per core
T = BL * S          # 16384 tokens per core
KT = 4              # contraction k-tiles of 128 (2 static + 2 dynamic)
MT = H3 // 128      # 6 output o-tiles
WSCALE = 32.0       # fp8 pre-scale on W12
ALPHA = 0.55        # linearization coefficient
F32 = mybir.dt.float32
BF16 = mybir.dt.bfloat16
FP8 = mybir.dt.float8e4
TANH = mybir.ActivationFunctionType.Tanh
DR = mybir.MatmulPerfMode.DoubleRow

_CACHED = {}


def build_bass():
    nc = bacc.Bacc(None, target_bir_lowering=False, debug=False)
    x = nc.dram_tensor("x8_t", [2 * H, T], FP8, kind="ExternalInput")
    xr = x.rearrange("(t p) n -> p t n", p=128)  # [128, 4, T]
    wt8 = nc.dram_tensor("wt8", [2 * H, H3], FP8, kind="ExternalInput")
    # host pre-transposes bias/v into per-partition-contiguous layouts so
    # their DMAs are a few large packets, not 768 tiny strided ones
    biasx = nc.dram_tensor("biasx", [128, MT * BL], F32, kind="ExternalInput")
    vv = nc.dram_tensor("v", [128, MT], BF16, kind="ExternalInput")
    # per b: 4 col strips x (two row-half partial scores, 512 tokens each);
    # host sums the halves, adds corr, softmaxes.
    out = nc.dram_tensor("out", [BL, 4, 1024], F32, kind="ExternalOutput")

    with tile.TileContext(nc) as tc:
        with (
            tc.tile_pool(name="const", bufs=1) as constp,
            tc.tile_pool(name="xp", bufs=2) as xp,
            tc.tile_pool(name="thp", bufs=15) as thp,
            tc.tile_pool(name="hps", bufs=4, space="PSUM") as hps,
        ):
            # ---- PE warmup: a few zero matmuls bridge the gap until the
            # first x/wt8 DMAs land (and start the HAM busy window).  The
            # warm tile is the first allocation of the h ring. ----
            warm = constp.tile([1, 512], BF16)
            nc.gpsimd.memset(warm, 0.0)
            warm_ps = hps.tile([128, 1024], F32, tag="h", name="warm_ps")
            for i in range(7):
                nc.tensor.matmul(
                    out=warm_ps[:, 0:512], lhsT=warm[:, 0:128], rhs=warm,
                    start=True, stop=True,
                )

            # ---- startup DMAs, balanced across both rings.  b0's x is
            # fetched as 4 whole k-tile stripes ([128, 1, 2048] each ->
            # 2048B contiguous lines, twice the packet efficiency of
            # token-chunk splits).  bias/v first (tiny, gate the first
            # tanh); wt8 k-tiles split 2+2.
            # wt8 is 32*W12.T quantized e4m3: [k 0..511, o]; k-tiles 0-1
            # static, 2-3 dynamic.
            wt8r = wt8.rearrange("(t p) o -> p t o", p=128)
            wt8_sb = constp.tile([128, KT, H3], FP8)
            first_xt = xp.tile([128, KT, S], FP8, tag="x", name="x_0")
            v_sb = constp.tile([128, MT], BF16)
            bias_sb = constp.tile([128, MT * BL], F32)

            # the scalar HWDGE ring measures ~2.5x slower than sync; keep all
            # startup bulk on sync, ordered by first-tanh dependency.  b0 x
            # moves in [kt-pair x token-half] quarters (1024B lines).
            nc.scalar.dma_start(out=bias_sb, in_=biasx[:, :])
            nc.scalar.dma_start(out=v_sb, in_=vv[:, :])
            nc.sync.dma_start(out=wt8_sb[:, 0:1, :], in_=wt8r[:, 0:1, :])
            nc.sync.dma_start(out=wt8_sb[:, 1:2, :], in_=wt8r[:, 1:2, :])
            nc.sync.dma_start(out=first_xt[:, 0:2, 0:1024], in_=xr[:, 0:2, 0:1024])
            nc.sync.dma_start(out=wt8_sb[:, 2:3, :], in_=wt8r[:, 2:3, :])
            nc.sync.dma_start(out=wt8_sb[:, 3:4, :], in_=wt8r[:, 3:4, :])
            nc.sync.dma_start(out=first_xt[:, 2:4, 0:1024], in_=xr[:, 2:4, 0:1024])
            nc.sync.dma_start(
                out=first_xt[:, 0:2, 1024:2048], in_=xr[:, 0:2, 1024:2048]
            )
            nc.sync.dma_start(
                out=first_xt[:, 2:4, 1024:2048], in_=xr[:, 2:4, 1024:2048]
            )

            # ---- main loop ----
            # v-dot for batch b is emitted after batch b+1's first matmul
            # block so the PE never idles waiting on b's last tanh
            pending = []

            def flush_pending():
                for emit in pending:
                    emit()
                pending.clear()

            next_xt = {}
            for b in range(BL):
                ths = {}
                if b == 0:
                    xt = first_xt
                else:
                    xt = next_xt.pop(b)
                for m in range(MT):
                    for hf in range(2):
                        if b > 0 and m == 1 and hf == 0:
                            flush_pending()
                        if m == 3 and hf == 0 and b + 1 < BL:
                            # prefetch next batch's x mid-batch: late enough
                            # not to starve startup/steady DMAs, early enough
                            # to land before b+1 needs it
                            nxt = xp.tile(
                                [128, KT, S], FP8, tag="x", name=f"x_{b + 1}"
                            )
                            nc.sync.dma_start(
                                out=nxt, in_=xr[:, :, (b + 1) * S : (b + 2) * S]
                            )
                            next_xt[b + 1] = nxt
                        h_ps = hps.tile(
                            [128, 1024], F32, tag="h", name=f"h_{b}_{m}_{hf}"
                        )
                        for c2 in range(2):
                            lo = (2 * hf + c2) * 512
                            for t_i in range(2):
                                nc.tensor.matmul(
                                    out=h_ps[:, c2 * 512 : (c2 + 1) * 512],
                                    lhsT=wt8_sb[:, 2 * t_i : 2 * t_i + 2,
                                                m * 128 : (m + 1) * 128],
                                    rhs=xt[:, 2 * t_i : 2 * t_i + 2, lo : lo + 512],
                                    start=(t_i == 0),
                                    stop=(t_i == 1),
                                    perf_mode=DR,
                                )
                        th = thp.tile(
                            [128, 1024], BF16, tag="tanh", name=f"th_{b}_{m}_{hf}"
                        )
                        nc.scalar.activation(
                            out=th, in_=h_ps, func=TANH,
                            scale=1.0 / WSCALE,
                            bias=bias_sb[:, m * BL + b : m * BL + b + 1],
                        )
                        ths[(m, hf)] = th

                def emit_scores(b=b, ths=ths):
                    # 2-row x 4-col array tiling: 8 concurrent K=64 tiles.
                    # Strip (a, c): contraction rows 64a..64a+63, output at
                    # partition 32c, free range 512a..512a+512.
                    score = hps.tile([128, 1024], F32, tag="h", name=f"sc_{b}")
                    for m in range(MT):
                        for a in range(2):
                            for c in range(4):
                                nc.tensor.matmul(
                                    out=score[32 * c : 32 * c + 1,
                                              512 * a : 512 * a + 512],
                                    lhsT=v_sb[64 * a : 64 * a + 64, m : m + 1],
                                    rhs=ths[(m, c // 2)][
                                        64 * a : 64 * a + 64,
                                        (c % 2) * 512 : (c % 2 + 1) * 512],
                                    start=(m == 0),
                                    stop=(m == MT - 1),
                                    tile_position=(64 * a, 32 * c),
                                )
                    # raw score strips PSUM -> SBUF (idle DVE, full-width --
                    # engines can't stride partitions) -> HBM (DMA can);
                    # host does half-sum + corr + softmax
                    stage = thp.tile(
                        [128, 1024], F32, tag="stage", bufs=2, name=f"stage_{b}"
                    )
                    if b == BL - 1:
                        # tail: split across DVE + ACT (ACT is idle after the
                        # last tanh) to shorten the drain chain
                        nc.vector.tensor_copy(stage[:, 0:512], score[:, 0:512])
                        nc.scalar.copy(stage[:, 512:1024], score[:, 512:1024])
                    else:
                        nc.vector.tensor_copy(stage, score)
                    sview = stage.rearrange("(c r) f -> c r f", c=4)[:, 0, :]
                    nc.scalar.dma_start(out=out[b], in_=sview)

                pending.append(emit_scores)
            flush_pending()

    nc.compile()
    return nc


def kernel(static, dynamic, decoder_hidden, v, W):
    static = np.ascontiguousarray(np.asarray(static, dtype=np.float32))
    dynamic = np.ascontiguousarray(np.asarray(dynamic, dtype=np.float32))
    decoder_hidden = np.ascontiguousarray(np.asarray(decoder_hidden, dtype=np.float32))
    v = np.ascontiguousarray(np.asarray(v, dtype=np.float32))
    W = np.ascontiguousarray(np.asarray(W, dtype=np.float32))

    bf16 = ml_dtypes.bfloat16
    e4m3 = ml_dtypes.float8_e4m3

    W12 = W[:, : 2 * H]                       # [768, 512]
    wt8 = np.ascontiguousarray(W12.T * WSCALE).astype(e4m3)   # [512, 768]
    # per-batch decoder bias (bf16 operands, f32 accumulate -- matches the
    # PE bias path of the bf16 baseline)
    W3f = W[:, 2 * H :].astype(bf16).astype(np.float32)        # [768, 256]
    decf = decoder_hidden.astype(bf16).astype(np.float32)      # [B, 256]
    bias_all = W3f @ decf.T                                    # [768, B]

    # linearization correction (host): ALPHA * (x @ u_lin - x8 @ u8)
    u_lin = W12.astype(np.float64).T @ v[0].astype(np.float64)          # [512]
    u8 = (wt8.astype(np.float64) / WSCALE) @ v[0].astype(np.float64)    # [512]

    in_maps = []
    corrs = []
    for c in range(NCORES):
        sl = slice(c * BL, (c + 1) * BL)
        xc = np.empty((T, 2 * H), dtype=np.float32)
        xc[:, :H] = static[sl].reshape(T, H)
        xc[:, H:] = dynamic[sl].reshape(T, H)
        x8 = xc.astype(e4m3)                         # [T, 512]
        x8f = x8.astype(np.float32)
        corr = ALPHA * (
            xc @ u_lin.astype(np.float32) - x8f @ u8.astype(np.float32)
        )                                            # [T]
        x8_t = np.ascontiguousarray(x8.T)            # [512, T]
        corrs.append(corr)
        # bias/v pre-transposed to per-partition-contiguous [128, m*BL+b]
        # and [128, m] layouts (o = m*128 + p)
        bias_pt = np.ascontiguousarray(
            bias_all[:, sl].reshape(MT, 128, BL).transpose(1, 0, 2).reshape(128, -1)
        )
        v_pt = np.ascontiguousarray(v[0].reshape(MT, 128).T.astype(bf16))
        in_maps.append({
            "x8_t": x8_t, "wt8": wt8,
            "biasx": bias_pt,
            "v": v_pt,
        })

    if "nc" not in _CACHED:
        _CACHED["nc"] = build_bass()
    nc = _CACHED["nc"]

    trace = bool(int(os.environ.get("KERNEL_TRACE", "0")))
    res = run_bass_kernel_spmd(
        nc, in_maps, core_ids=list(range(NCORES)), trace=trace,
        trace_cores=[0] if trace else None,
    )
    _CACHED["last_result"] = res

    raw = np.concatenate([r["out"] for r in res.results], axis=0)  # [B, 4, 1024]
    s = raw[:, :, :512].astype(np.float64) + raw[:, :, 512:].astype(np.float64)
    z = s.reshape(B, S) + np.concatenate(corrs).reshape(B, S).astype(np.float64)
    z -= z.max(axis=1, keepdims=True)
    ez = np.exp(z)
    return (ez / ez.sum(axis=1, keepdims=True)).astype(np.float32)


# revision 19
# speedup vs baseline: 1.0356x; 1.0076x over previous
"""Trainium2 Bass kernel for nn_Attention_27977416966318 (sparse_attention).

score[b,s] = v . tanh(W @ concat(static[b,s], dynamic[b,s], dec[b]))
out = softmax(score, axis=1)

Shapes: static/dynamic [64, 2048, 256] f32, decoder_hidden [64, 256],
v [1, 768], W [768, 768].  Output [64, 2048] f32.

Strategy: data-parallel over batch B=64 across 8 NeuronCores (8 batches
per core).  The dominant matmul W12 @ [static; dynamic] runs in fp8
(e4m3) with DoubleRow perf mode.  W12 is pre-scaled by 32 before fp8
quantization; the tanh activation applies scale=1/32 to undo it and
adds the per-batch decoder bias (host-computed W3 @ dec).

The kernel is dual-bottlenecked on PE (fp8-DR matmul stream) and ACT
(tanh on 768*2048 elems/batch).  vs the previous version:
  - tanh tiles are [128, 2048] (PSUM pair of h buffers = all 8 banks),
    halving the ~540-cycle per-ACTIVATE overhead contribution.
  - exp moved to the host (device ships raw score strips; softmax is
    computed host-side in f64) - saves ACT time and the tail.
  - the v-dot uses 2-row x 4-col array tiling (8 concurrent K=64
    tiles); the two row-half partial scores are summed on the host.
  - the score accumulator rides the h PSUM pool ring (same tag), so
    no PSUM bank is reserved for it.

fp8 quantization error is tamed by a rank-1 linearization correction
computed on the host:

    corr = ALPHA * ((v^T W12) @ x  -  (v^T W8/32) @ x8)

where x8/W8 are the exact fp8 operand values the PE sees.  Writing
score = v.tanh(h~) + ALPHA*v.(h - h~), the residual sensitivity is
(tanh'(h) - ALPHA); ALPHA=0.55 ~ E[tanh'] minimizes it (rel err
~1.25e-2, gate 2e-2).  The host adds corr to the raw scores before
its softmax.
"""

import os

import numpy as np
import ml_dtypes

import concourse.bass as bass
from concourse import bacc
import concourse.mybir as mybir
import concourse.tile as tile
from concourse.bass_utils import run_bass_kernel_spmd

B, S, H = 64, 2048, 256
H3 = 3 * H          # 768
NCORES = 8
BL = B // NCORES    # 8 batches per core
T = BL * S          # 16384 tokens per core
KT = 4              # contraction k-tiles of 128 (2 static + 2 dynamic)
MT = H3 // 128      # 6 output o-tiles
WSCALE = 32.0       # fp8 pre-scale on W12
ALPHA = 0.55        # linearization coefficient
F32 = mybir.dt.float32
BF16 = mybir.dt.bfloat16
FP8 = mybir.dt.float8e4
TANH = mybir.ActivationFunctionType.Tanh
DR = mybir.MatmulPerfMode.DoubleRow

_CACHED = {}


def build_bass():
    nc = bacc.Bacc(None, target_bir_lowering=False, debug=False)
    x = nc.dram_tensor("x8_t", [2 * H, T], FP8, kind="ExternalInput")
    xr = x.rearrange("(t p) n -> p t n", p=128)  # [128, 4, T]
    wt8 = nc.dram_tensor("wt8", [2 * H, H3], FP8, kind="ExternalInput")
    # host pre-transposes bias/v into per-partition-contiguous layouts so
    # their DMAs are a few large packets, not 768 tiny strided ones
    biasx = nc.dram_tensor("biasx", [128, MT * BL], F32, kind="ExternalInput")
    vv = nc.dram_tensor("v", [128, MT], BF16, kind="ExternalInput")
    # per b: 4 col strips x (two row-half partial scores, 512 tokens each);
    # host sums the halves, adds corr, softmaxes.
    out = nc.dram_tensor("out", [BL, 4, 1024], F32, kind="ExternalOutput")

    with tile.TileContext(nc) as tc:
        with (
            tc.tile_pool(name="const", bufs=1) as constp,
            tc.tile_pool(name="xp", bufs=2) as xp,
            tc.tile_pool(name="thp", bufs=15) as thp,
            tc.tile_pool(name="hps", bufs=4, space="PSUM") as hps,
        ):
            # ---- PE warmup: a few zero matmuls bridge the gap until the
            # first x/wt8 DMAs land (and start the HAM busy window).  The
            # warm tile is the first allocation of the h ring. ----
            warm = constp.tile([1, 512], BF16)
            nc.gpsimd.memset(warm, 0.0)
            warm_ps = hps.tile([128, 1024], F32, tag="h", name="warm_ps")
            for i in range(7):
                nc.tensor.matmul(
                    out=warm_ps[:, 0:512], lhsT=warm[:, 0:128], rhs=warm,
                    start=True, stop=True,
                )

            # ---- startup DMAs, balanced across both rings.  b0's x is
            # fetched as 4 whole k-tile stripes ([128, 1, 2048] each ->
            # 2048B contiguous lines, twice the packet efficiency of
            # token-chunk splits).  bias/v first (tiny, gate the first
            # tanh); wt8 k-tiles split 2+2.
            # wt8 is 32*W12.T quantized e4m3: [k 0..511, o]; k-tiles 0-1
            # static, 2-3 dynamic.
            wt8r = wt8.rearrange("(t p) o -> p t o", p=128)
            wt8_sb = constp.tile([128, KT, H3], FP8)
            first_xt = xp.tile([128, KT, S], FP8, tag="x", name="x_0")
            v_sb = constp.tile([128, MT], BF16)
            bias_sb = constp.tile([128, MT * BL], F32)

            # the scalar HWDGE ring measures ~2.5x slower than sync; keep all
            # startup bulk on sync, ordered by first-tanh dependency.  b0 x
            # moves in [kt-pair x token-half] quarters (1024B lines).
            nc.scalar.dma_start(out=bias_sb, in_=biasx[:, :])
            nc.scalar.dma_start(out=v_sb, in_=vv[:, :])
            nc.sync.dma_start(out=wt8_sb[:, 0:1, :], in_=wt8r[:, 0:1, :])
            nc.sync.dma_start(out=wt8_sb[:, 1:2, :], in_=wt8r[:, 1:2, :])
            nc.sync.dma_start(out=first_xt[:, 0:2, 0:1024], in_=xr[:, 0:2, 0:1024])
            nc.sync.dma_start(out=wt8_sb[:, 2:3, :], in_=wt8r[:, 2:3, :])
            nc.sync.dma_start(out=wt8_sb[:, 3:4, :], in_=wt8r[:, 3:4, :])
            nc.sync.dma_start(out=first_xt[:, 2:4, 0:1024], in_=xr[:, 2:4, 0:1024])
            nc.sync.dma_start(
                out=first_xt[:, 0:2, 1024:2048], in_=xr[:, 0:2, 1024:2048]
            )
            # D rides the (slower) scalar ring concurrently; it only gates
            # the second tanh
            nc.scalar.dma_start(
                out=first_xt[:, 2:4, 1024:2048], in_=xr[:, 2:4, 1024:2048]
            )

            # ---- main loop ----
            # v-dot for batch b is emitted after batch b+1's first matmul
            # block so the PE never idles waiting on b's last tanh
            pending = []

            def flush_pending():
                for emit in pending:
                    emit()
                pending.clear()

            next_xt = {}
            for b in range(BL):
                ths = {}
                if b == 0:
                    xt = first_xt
                else:
                    xt = next_xt.pop(b)
                for m in range(MT):
                    for hf in range(2):
                        if b > 0 and m == 1 and hf == 0:
                            flush_pending()
                        if m == 3 and hf == 0 and b + 1 < BL:
                            # prefetch next batch's x mid-batch: late enough
                            # not to starve startup/steady DMAs, early enough
                            # to land before b+1 needs it
                            nxt = xp.tile(
                                [128, KT, S], FP8, tag="x", name=f"x_{b + 1}"
                            )
                            nc.sync.dma_start(
                                out=nxt, in_=xr[:, :, (b + 1) * S : (b + 2) * S]
                            )
                            next_xt[b + 1] = nxt
                        h_ps = hps.tile(
                            [128, 1024], F32, tag="h", name=f"h_{b}_{m}_{hf}"
                        )
                        for c2 in range(2):
                            lo = (2 * hf + c2) * 512
                            for t_i in range(2):
                                nc.tensor.matmul(
                                    out=h_ps[:, c2 * 512 : (c2 + 1) * 512],
                                    lhsT=wt8_sb[:, 2 * t_i : 2 * t_i + 2,
                                                m * 128 : (m + 1) * 128],
                                    rhs=xt[:, 2 * t_i : 2 * t_i + 2, lo : lo + 512],
                                    start=(t_i == 0),
                                    stop=(t_i == 1),
                                    perf_mode=DR,
                                )
                        th = thp.tile(
                            [128, 1024], BF16, tag="tanh", name=f"th_{b}_{m}_{hf}"
                        )
                        nc.scalar.activation(
                            out=th, in_=h_ps, func=TANH,
                            scale=1.0 / WSCALE,
                            bias=bias_sb[:, m * BL + b : m * BL + b + 1],
                        )
                        ths[(m, hf)] = th

                def emit_scores(b=b, ths=ths):
                    # 2-row x 4-col array tiling: 8 concurrent K=64 tiles.
                    # Strip (a, c): contraction rows 64a..64a+63, output at
                    # partition 32c, free range 512a..512a+512.
                    score = hps.tile([128, 1024], F32, tag="h", name=f"sc_{b}")
                    for m in range(MT):
                        for a in range(2):
                            for c in range(4):
                                nc.tensor.matmul(
                                    out=score[32 * c : 32 * c + 1,
                                              512 * a : 512 * a + 512],
                                    lhsT=v_sb[64 * a : 64 * a + 64, m : m + 1],
                                    rhs=ths[(m, c // 2)][
                                        64 * a : 64 * a + 64,
                                        (c % 2) * 512 : (c % 2 + 1) * 512],
                                    start=(m == 0),
                                    stop=(m == MT - 1),
                                    tile_position=(64 * a, 32 * c),
                                )
                    # raw score strips PSUM -> SBUF (idle DVE, full-width --
                    # engines can't stride partitions) -> HBM (DMA can);
                    # host does half-sum + corr + softmax
                    stage = thp.tile(
                        [128, 1024], F32, tag="stage", bufs=2, name=f"stage_{b}"
                    )
                    if b == BL - 1:
                        # tail: split across DVE + ACT (ACT is idle after the
                        # last tanh) to shorten the drain chain
                        nc.vector.tensor_copy(stage[:, 0:512], score[:, 0:512])
                        nc.scalar.copy(stage[:, 512:1024], score[:, 512:1024])
                    else:
                        nc.vector.tensor_copy(stage, score)
                    sview = stage.rearrange("(c r) f -> c r f", c=4)[:, 0, :]
                    nc.scalar.dma_start(out=out[b], in_=sview)

                pending.append(emit_scores)
            flush_pending()

    nc.compile()
    return nc


def kernel(static, dynamic, decoder_hidden, v, W):
    static = np.ascontiguousarray(np.asarray(static, dtype=np.float32))
    dynamic = np.ascontiguousarray(np.asarray(dynamic, dtype=np.float32))
    decoder_hidden = np.ascontiguousarray(np.asarray(decoder_hidden, dtype=np.float32))
    v = np.ascontiguousarray(np.asarray(v, dtype=np.float32))
    W = np.ascontiguousarray(np.asarray(W, dtype=np.float32))

    bf16 = ml_dtypes.bfloat16
    e4m3 = ml_dtypes.float8_e4m3

    W12 = W[:, : 2 * H]                       # [768, 512]
    wt8 = np.ascontiguousarray(W12.T * WSCALE).astype(e4m3)   # [512, 768]
    # per-batch decoder bias (bf16 operands, f32 accumulate -- matches the
    # PE bias path of the bf16 baseline)
    W3f = W[:, 2 * H :].astype(bf16).astype(np.float32)        # [768, 256]
    decf = decoder_hidden.astype(bf16).astype(np.float32)      # [B, 256]
    bias_all = W3f @ decf.T                                    # [768, B]

    # linearization correction (host): ALPHA * (x @ u_lin - x8 @ u8)
    u_lin = W12.astype(np.float64).T @ v[0].astype(np.float64)          # [512]
    u8 = (wt8.astype(np.float64) / WSCALE) @ v[0].astype(np.float64)    # [512]

    in_maps = []
    corrs = []
    for c in range(NCORES):
        sl = slice(c * BL, (c + 1) * BL)
        xc = np.empty((T, 2 * H), dtype=np.float32)
        xc[:, :H] = static[sl].reshape(T, H)
        xc[:, H:] = dynamic[sl].reshape(T, H)
        x8 = xc.astype(e4m3)                         # [T, 512]
        x8f = x8.astype(np.float32)
        corr = ALPHA * (
            xc @ u_lin.astype(np.float32) - x8f @ u8.astype(np.float32)
        )                                            # [T]
        x8_t = np.ascontiguousarray(x8.T)            # [512, T]
        corrs.append(corr)
        # bias/v pre-transposed to per-partition-contiguous [128, m*BL+b]
        # and [128, m] layouts (o = m*128 + p)
        bias_pt = np.ascontiguousarray(
            bias_all[:, sl].reshape(MT, 128, BL).transpose(1, 0, 2).reshape(128, -1)
        )
        v_pt = np.ascontiguousarray(v[0].reshape(MT, 128).T.astype(bf16))
        in_maps.append({
            "x8_t": x8_t, "wt8": wt8,
            "biasx": bias_pt,
            "v": v_pt,
        })

    if "nc" not in _CACHED:
        _CACHED["nc"] = build_bass()
    nc = _CACHED["nc"]

    trace = bool(int(os.environ.get("KERNEL_TRACE", "0")))
    res = run_bass_kernel_spmd(
        nc, in_maps, core_ids=list(range(NCORES)), trace=trace,
        trace_cores=[0] if trace else None,
    )
    _CACHED["last_result"] = res

    raw = np.concatenate([r["out"] for r in res.results], axis=0)  # [B, 4, 1024]
    s = raw[:, :, :512].astype(np.float64) + raw[:, :, 512:].astype(np.float64)
    z = s.reshape(B, S) + np.concatenate(corrs).reshape(B, S).astype(np.float64)
    z -= z.max(axis=1, keepdims=True)
    ez = np.exp(z)
    return (ez / ez.sum(axis=1, keepdims=True)).astype(np.float32)
